# revision 3
# baseline (speedup 1.0000x reference)
"""Trainium2 Bass kernel for nn_DynamicRangeCompressor.

Input : audio [16, 1, 2097152] f32 (+ scalar params threshold/ratio/makeup/
        attack_time/release_time as [1] arrays).
Output: [16, 1, 2097152] f32.

Sharding: pure data parallel — 2 batch rows per core across 8 NeuronCores.

Algorithm restructuring (validated vs reference to ~3e-7 rel err):
- Work in natural-log units (U = dB * ln10/20 + makeup_nat) so Ln/Exp replace
  log10/10**x and all scale factors fold away.
- linear_downsample(DS=16) == 0.5*(g[16i+7]+g[16i+8]): only 2/16 gain taps.
- The attack/release one-pole recurrence is branch-linearized: the coefficient
  is chosen by comparing gd[t] >= gd[t-1] instead of gd[t] >= y[t-1]. Since
  the coefficients are ~5e-5 the state tracks the target to ~3e-3 dB and the
  substitution changes the output by <1e-6 dB.  The scan becomes a LINEAR
  first-order IIR y = c[t]*y + b[t], which runs at stream rate on the DVE via
  tensor_tensor_scan(mult, add).
- Partition-parallel scan: each of 128 partitions scans its own time segment,
  seeded with W=4 warmup frames from the preceding segment (coefficient
  contraction (5.5e-5)^4 makes segments exactly independent in fp32).
- Hann overlap-add upsample == per-frame lerp: L[16q+r] = U[q] + dU[q]*w0[r],
  emitted as 16 strided scalar_tensor_tensor ops.
- out = audio * exp(L)  (drops reference's sign(a)*1e-8 term: |err| <= 1.5e-8).
"""
import os
import sys

for _p in ("/opt/trn_rl_repo", "/opt/pypackages"):
    if _p not in sys.path and os.path.isdir(_p):
        sys.path.append(_p)

import math
import numpy as np

import concourse.bass as bass
import concourse.tile as tile
from concourse import bacc, mybir
from concourse.bass_utils import run_bass_kernel_spmd

# problem constants (hardcoded per spec)
B_TOTAL = 16
T = 2097152
N_CORES = 8
NCH = 2               # batch rows per core
P = 128               # SBUF partitions
FD = T // P           # 16384 free-dim samples per partition per channel
S = 4                 # chunks per channel
M = FD // S           # samples per partition per chunk per channel
G = M // 16           # frames per partition per chunk per channel
W = 4                 # scan warmup frames
CS = P * M            # samples per chunk per channel
GW = G + W

F32 = mybir.dt.float32
OP = mybir.AluOpType
AF = mybir.ActivationFunctionType

LAST_RESULTS = None   # stashed BassKernelResults for test harness introspection


def _build(thr, ratio, makeup, at, rt):
    ln10_20 = math.log(10.0) / 20.0
    thr_nat = float(np.float32(thr * ln10_20))
    mk_nat = float(np.float32(makeup * ln10_20))
    gscale = float(np.float32(-(1.0 - 1.0 / ratio) / 2.0))   # -0.375
    at = float(np.float32(at))
    rt = float(np.float32(rt))
    w0 = [float(0.5 * (1.0 - math.cos(2.0 * math.pi * r / 32.0))) for r in range(16)]

    nc = bacc.Bacc("TRN2", target_bir_lowering=False, debug=False)
    audio = nc.dram_tensor("audio", [NCH, T], F32, kind="ExternalInput")
    out = nc.dram_tensor("out", [NCH, T], F32, kind="ExternalOutput")

    def hbm_view(dram, c, s):
        return dram[c:c + 1, s * CS:(s + 1) * CS].rearrange(
            "one (p m) -> (one p) m", p=P)

    with tile.TileContext(nc) as tc:
        with tc.tile_pool(name="big", bufs=2) as pb, \
             tc.tile_pool(name="fr", bufs=2) as pf, \
             tc.tile_pool(name="consts", bufs=1) as pc:

            bias_eps = pc.tile([P, 1], F32, tag="bias_eps")
            bias_nthr = pc.tile([P, 1], F32, tag="bias_nthr")
            nc.vector.memset(bias_eps[:], 1e-8)
            nc.vector.memset(bias_nthr[:], -thr_nat)

            st = [{} for _ in range(S)]  # per-chunk tiles

            def prep_scan(s):
                d = st[s]
                A = pb.tile([P, 2 * M], F32, tag="A")
                nc.sync.dma_start(out=A[:, 0:M], in_=hbm_view(audio, 0, s))
                nc.sync.dma_start(out=A[:, M:2 * M], in_=hbm_view(audio, 1, s))

                t7 = pf.tile([P, 2 * G], F32, tag="t7")
                t8 = pf.tile([P, 2 * G], F32, tag="t8")
                for t_, off in ((t7, 7), (t8, 8)):
                    nc.scalar.activation(t_[:], A[:, off::16], AF.Abs)
                    nc.scalar.activation(t_[:], t_[:], AF.Ln, bias=bias_eps[:])
                    nc.scalar.activation(t_[:], t_[:], AF.Relu, bias=bias_nthr[:])
                # gd (nat units incl makeup)
                gdf = pf.tile([P, 2 * G], F32, tag="gdf")
                nc.vector.tensor_tensor(out=t7[:], in0=t7[:], in1=t8[:], op=OP.add)
                nc.vector.tensor_scalar(out=gdf[:], in0=t7[:], scalar1=gscale,
                                        scalar2=mk_nat, op0=OP.mult, op1=OP.add)
                # branch mask m: gd[t] >= gd[t-1]
                gprev = pf.tile([P, 2], F32, tag="gprev")
                nc.sync.dma_start(out=gprev[1:P, 0:2], in_=gdf[0:P - 1, G - 1::G])
                if s > 0:
                    nc.sync.dma_start(out=gprev[0:1, 0:2],
                                      in_=st[s - 1]["gdf"][P - 1:P, G - 1::G])
                else:
                    nc.sync.dma_start(out=gprev[0:1, 0:2], in_=gdf[0:1, 0::G])
                mt = pf.tile([P, 2 * G], F32, tag="mt")
                nc.vector.tensor_tensor(out=mt[:, 1:2 * G], in0=gdf[:, 1:2 * G],
                                        in1=gdf[:, 0:2 * G - 1], op=OP.is_ge)
                nc.vector.tensor_tensor(out=mt[:, 0::G], in0=gdf[:, 0::G],
                                        in1=gprev[:, 0:2], op=OP.is_ge)
                # coefficients into ext arrays (ch0: [0,G+W), ch1: [G+W, 2G+2W))
                cext = pf.tile([P, 2 * GW], F32, tag="cext")
                bext = pf.tile([P, 2 * GW], F32, tag="bext")
                cv = cext[:].rearrange("p (c gw) -> p c gw", c=2)
                bv = bext[:].rearrange("p (c gw) -> p c gw", c=2)
                mv = mt[:].rearrange("p (c g) -> p c g", c=2)
                gv = gdf[:].rearrange("p (c g) -> p c g", c=2)
                nc.vector.tensor_scalar(out=cv[:, :, W:GW], in0=mv[:], scalar1=at - rt,
                                        scalar2=rt, op0=OP.mult, op1=OP.add)
                nc.vector.tensor_scalar(out=mt[:], in0=mt[:], scalar1=rt - at,
                                        scalar2=1.0 - rt, op0=OP.mult, op1=OP.add)
                nc.vector.tensor_tensor(out=bv[:, :, W:GW], in0=mv[:], in1=gv[:],
                                        op=OP.mult)
                # warmup columns (last W real frames of preceding segment)
                nc.sync.dma_start(out=cv[1:P, :, 0:W], in_=cv[0:P - 1, :, G:GW])
                nc.sync.dma_start(out=bv[1:P, :, 0:W], in_=bv[0:P - 1, :, G:GW])
                if s > 0:
                    pcv = st[s - 1]["cext"][:].rearrange("p (c gw) -> p c gw", c=2)
                    pbv = st[s - 1]["bext"][:].rearrange("p (c gw) -> p c gw", c=2)
                    nc.sync.dma_start(out=cv[0:1, :, 0:W], in_=pcv[P - 1:P, :, G:GW])
                    nc.sync.dma_start(out=bv[0:1, :, 0:W], in_=pbv[P - 1:P, :, G:GW])
                else:
                    nc.vector.memset(cv[0:1, :, 0:W], 1.0)
                    nc.vector.memset(bv[0:1, :, 0:W], 0.0)
                # the scan (one linear IIR per partition per channel)
                U = pf.tile([P, 2 * GW], F32, tag="U")
                for c in range(2):
                    lo, hi = c * GW, (c + 1) * GW
                    nc.vector.tensor_tensor_scan(
                        out=U[:, lo:hi], data0=cext[:, lo:hi], data1=bext[:, lo:hi],
                        initial=gdf[:, c * G:c * G + 1], op0=OP.mult, op1=OP.add)
                # dU within segment
                du = pf.tile([P, 2 * G], F32, tag="du")
                uv = U[:].rearrange("p (c gw) -> p c gw", c=2)
                dv = du[:].rearrange("p (c g) -> p c g", c=2)
                nc.vector.tensor_tensor(out=dv[:, :, 0:G - 1], in0=uv[:, :, W + 1:W + G],
                                        in1=uv[:, :, W:W + G - 1], op=OP.subtract)
                d.update(A=A, gdf=gdf, cext=cext, bext=bext, U=U, du=du)

            def apply(s):
                d = st[s]
                U, du, A = d["U"], d["du"], d["A"]
                uv = U[:].rearrange("p (c gw) -> p c gw", c=2)
                # next-segment first U value (for the lerp across segment ends)
                unx = pf.tile([P, 2], F32, tag="unx")
                nc.sync.dma_start(out=unx[0:P - 1, 0:2], in_=U[1:P, W::GW])
                if s < S - 1:
                    nc.sync.dma_start(out=unx[P - 1:P, 0:2],
                                      in_=st[s + 1]["U"][0:1, W::GW])
                else:
                    nc.sync.dma_start(out=unx[P - 1:P, 0:2],
                                      in_=U[P - 1:P, W + G - 1::GW])
                nc.vector.tensor_tensor(out=du[:, G - 1::G], in0=unx[:, 0:2],
                                        in1=U[:, W + G - 1::GW], op=OP.subtract)
                # upsample lerp: L[p, c, 16g+r] = U[g] + dU[g]*w0[r]
                L = pb.tile([P, 2 * M], F32, tag="L")
                lv = L[:].rearrange("p (c m) -> p c m", c=2)
                dv = du[:].rearrange("p (c g) -> p c g", c=2)
                for r in range(16):
                    nc.vector.scalar_tensor_tensor(
                        out=lv[:, :, r::16], in0=dv[:], scalar=w0[r],
                        in1=uv[:, :, W:W + G], op0=OP.mult, op1=OP.add)
                nc.scalar.activation(L[:], L[:], AF.Exp)
                nc.vector.tensor_tensor(out=L[:], in0=A[:], in1=L[:], op=OP.mult)
                nc.sync.dma_start(out=hbm_view(out, 0, s), in_=L[:, 0:M])
                nc.sync.dma_start(out=hbm_view(out, 1, s), in_=L[:, M:2 * M])

            for s in range(S):
                prep_scan(s)
                if s > 0:
                    apply(s - 1)
            apply(S - 1)

    nc.compile()
    return nc


def kernel(audio, threshold, ratio, makeup, attack_time, release_time):
    global LAST_RESULTS
    a = np.asarray(audio, dtype=np.float32)
    B, C, Tin = a.shape
    assert (B, C, Tin) == (B_TOTAL, 1, T), (B, C, Tin)
    thr = float(np.asarray(threshold).ravel()[0])
    rat = float(np.asarray(ratio).ravel()[0])
    mk = float(np.asarray(makeup).ravel()[0])
    at = float(np.asarray(attack_time).ravel()[0])
    rt = float(np.asarray(release_time).ravel()[0])

    nc = _build(thr, rat, mk, at, rt)

    flat = a.reshape(B_TOTAL, T)
    in_maps = [{"audio": np.ascontiguousarray(flat[i * NCH:(i + 1) * NCH])}
               for i in range(N_CORES)]
    res = run_bass_kernel_spmd(nc, in_maps, list(range(N_CORES)))
    LAST_RESULTS = res
    outp = np.concatenate([res.results[i]["out"] for i in range(N_CORES)], axis=0)
    return outp.reshape(B_TOTAL, 1, T).astype(np.float32)


# revision 4
# speedup vs baseline: 1.0808x; 1.0808x over previous
"""Trainium2 Bass kernel for nn_DynamicRangeCompressor.

Input : audio [16, 1, 2097152] f32 (+ scalar params threshold/ratio/makeup/
        attack_time/release_time as [1] arrays).
Output: [16, 1, 2097152] f32.

Sharding: pure data parallel - 2 batch rows per core across 8 NeuronCores.

Algorithm restructuring (validated vs reference to ~3e-7 rel err):
- Work in natural-log units (U = dB * ln10/20 + makeup_nat) so Ln/Exp replace
  log10/10**x and all scale factors fold away.
- linear_downsample(DS=16) == 0.5*(g[16i+7]+g[16i+8]): only 2/16 gain taps.
- The attack/release one-pole recurrence is branch-linearized: the coefficient
  is chosen by comparing gd[t] >= gd[t-1] instead of gd[t] >= y[t-1]. Since
  the coefficients are ~5e-5 the state tracks the target to ~3e-3 dB and the
  substitution changes the output by <1e-6 dB.  The scan becomes a LINEAR
  first-order IIR y = c[t]*y + b[t], which runs at stream rate on the DVE via
  tensor_tensor_scan(mult, add).
- Partition-parallel scan: each of 128 partitions scans its own time segment,
  seeded with W=4 warmup frames from the preceding segment (coefficient
  contraction (5.5e-5)^4 makes segments exactly independent in fp32).
- Hann overlap-add upsample == per-frame lerp: L[16q+r] = U[q] + dU[q]*w0[r],
  emitted as 16 strided scalar_tensor_tensor ops.
- out = audio * exp(L)  (drops reference's sign(a)*1e-8 term: |err| <= 1.5e-8).
"""
import os
import sys

for _p in ("/opt/trn_rl_repo", "/opt/pypackages"):
    if _p not in sys.path and os.path.isdir(_p):
        sys.path.append(_p)

import math
import numpy as np

import concourse.bass as bass
import concourse.tile as tile
from concourse import bacc, mybir
from concourse.bass_utils import run_bass_kernel_spmd

# problem constants (hardcoded per spec)
B_TOTAL = 16
T = 2097152
N_CORES = 8
NCH = 2               # batch rows per core
P = 128               # SBUF partitions
FD = T // P           # 16384 free-dim samples per partition per channel
S = 4                 # chunks per channel
M = FD // S           # samples per partition per chunk per channel
G = M // 16           # frames per partition per chunk per channel
W = 4                 # scan warmup frames
CS = P * M            # samples per chunk per channel
GW = G + W

F32 = mybir.dt.float32
OP = mybir.AluOpType
AF = mybir.ActivationFunctionType

LAST_RESULTS = None   # stashed BassKernelResults for test harness introspection

# Pin all activations to the one table set that contains Abs/Ln/Relu/Exp/
# Identity together (natural_log_exp_and_others); the default greedy set
# selection alternates between two sets and reloads tables 7x per run.
import concourse.bacc as _bacc_mod
from concourse.hw_specs import get_activation_tables as _real_gat


def _gat_pinned(arch):
    real = _real_gat(arch)
    return {name: (fns if name == "natural_log_exp_and_others" else set())
            for name, fns in real.items()}


_bacc_mod.get_activation_tables = _gat_pinned


def _build(thr, ratio, makeup, at, rt):
    ln10_20 = math.log(10.0) / 20.0
    thr_nat = float(np.float32(thr * ln10_20))
    mk_nat = float(np.float32(makeup * ln10_20))
    gscale = float(np.float32(-(1.0 - 1.0 / ratio) / 2.0))   # -0.375
    at = float(np.float32(at))
    rt = float(np.float32(rt))
    w0 = [float(0.5 * (1.0 - math.cos(2.0 * math.pi * r / 32.0))) for r in range(16)]

    nc = bacc.Bacc("TRN2", target_bir_lowering=False, debug=False)
    audio = nc.dram_tensor("audio", [NCH, T], F32, kind="ExternalInput")
    out = nc.dram_tensor("out", [NCH, T], F32, kind="ExternalOutput")

    def hbm_pair(dram, s):
        # [128, 2, M]: partition-major chunk, both channels in one DMA
        return dram[:, s * CS:(s + 1) * CS].rearrange("c (p m) -> p c m", p=P)

    def hbm_one(dram, c, s):
        return dram[c:c + 1, s * CS:(s + 1) * CS].rearrange(
            "one (p m) -> (one p) m", p=P)

    with tile.TileContext(nc) as tc:
        with tc.tile_pool(name="big", bufs=2) as pb, \
             tc.tile_pool(name="fr", bufs=2) as pf, \
             tc.tile_pool(name="consts", bufs=1) as pc:

            bias_eps = pc.tile([P, 1], F32, tag="bias_eps")
            bias_nthr = pc.tile([P, 1], F32, tag="bias_nthr")
            bias_mk = pc.tile([P, 1], F32, tag="bias_mk")
            bias_rt = pc.tile([P, 1], F32, tag="bias_rt")
            bias_omrt = pc.tile([P, 1], F32, tag="bias_omrt")
            nc.vector.memset(bias_eps[:], 1e-8)
            nc.vector.memset(bias_nthr[:], -thr_nat)
            nc.vector.memset(bias_mk[:], mk_nat)
            nc.vector.memset(bias_rt[:], rt)
            nc.vector.memset(bias_omrt[:], 1.0 - rt)

            st = [{} for _ in range(S)]  # per-chunk tiles

            def prep_scan(s):
                d = st[s]
                A = pb.tile([P, 2 * M], F32, tag="A")
                av = A[:].rearrange("p (c m) -> p c m", c=2)
                nc.sync.dma_start(out=av[:], in_=hbm_pair(audio, s))

                t7 = pf.tile([P, 2 * G], F32, tag="t7")
                t8 = pf.tile([P, 2 * G], F32, tag="t8")
                for t_, off in ((t7, 7), (t8, 8)):
                    nc.scalar.activation(t_[:], A[:, off::16], AF.Abs)
                    nc.scalar.activation(t_[:], t_[:], AF.Ln, bias=bias_eps[:])
                    nc.scalar.activation(t_[:], t_[:], AF.Relu, bias=bias_nthr[:])
                # gd in nat units incl makeup: gdf = gscale*(t7+t8) + mk_nat
                nc.scalar.activation(t7[:], t7[:], AF.Identity, bias=bias_mk[:],
                                     scale=gscale)
                gdf = pf.tile([P, 2 * G], F32, tag="gdf")
                nc.vector.scalar_tensor_tensor(
                    out=gdf[:], in0=t8[:], scalar=gscale, in1=t7[:],
                    op0=OP.mult, op1=OP.add)
                # branch mask m: gd[t] >= gd[t-1]
                gprev = pf.tile([P, 2], F32, tag="gprev")
                nc.scalar.dma_start(out=gprev[1:P, 0:2], in_=gdf[0:P - 1, G - 1::G])
                if s > 0:
                    nc.scalar.dma_start(out=gprev[0:1, 0:2],
                                        in_=st[s - 1]["gdf"][P - 1:P, G - 1::G])
                else:
                    nc.scalar.dma_start(out=gprev[0:1, 0:2], in_=gdf[0:1, 0::G])
                mt = pf.tile([P, 2 * G], F32, tag="mt")
                nc.vector.tensor_tensor(out=mt[:, 1:2 * G], in0=gdf[:, 1:2 * G],
                                        in1=gdf[:, 0:2 * G - 1], op=OP.is_ge)
                nc.vector.tensor_tensor(out=mt[:, 0::G], in0=gdf[:, 0::G],
                                        in1=gprev[:, 0:2], op=OP.is_ge)
                # coefficients, written into the c|b ext array
                # layout: [c_ch0 | c_ch1 | b_ch0 | b_ch1], each GW wide
                cb = pf.tile([P, 4 * GW], F32, tag="cb")
                cbv = cb[:].rearrange("p (h gw) -> p h gw", h=4)
                mv = mt[:].rearrange("p (c g) -> p c g", c=2)
                gv = gdf[:].rearrange("p (c g) -> p c g", c=2)
                # c = m*(at-rt) + rt   (ACT)
                nc.scalar.activation(cbv[:, 0:2, W:GW], mt[:].rearrange(
                    "p (c g) -> p c g", c=2), AF.Identity, bias=bias_rt[:],
                    scale=at - rt)
                # omc = m*(rt-at) + (1-rt)   (ACT, in-place over m)
                nc.scalar.activation(mt[:], mt[:], AF.Identity,
                                     bias=bias_omrt[:], scale=rt - at)
                # b = omc * gd
                nc.vector.tensor_tensor(out=cbv[:, 2:4, W:GW], in0=mv[:], in1=gv[:],
                                        op=OP.mult)
                # warmup columns = last W real frames of the preceding segment
                nc.scalar.dma_start(out=cbv[1:P, :, 0:W], in_=cbv[0:P - 1, :, G:GW])
                if s > 0:
                    pcb = st[s - 1]["cb"][:].rearrange("p (h gw) -> p h gw", h=4)
                    nc.scalar.dma_start(out=cbv[0:1, :, 0:W],
                                        in_=pcb[P - 1:P, :, G:GW])
                else:
                    nc.vector.memset(cbv[0:1, 0:2, 0:W], 1.0)
                    nc.vector.memset(cbv[0:1, 2:4, 0:W], 0.0)
                # the scan: one linear IIR per partition per channel
                U = pf.tile([P, 2 * GW], F32, tag="U")
                for c in range(2):
                    nc.vector.tensor_tensor_scan(
                        out=U[:, c * GW:(c + 1) * GW],
                        data0=cb[:, c * GW:(c + 1) * GW],
                        data1=cb[:, (2 + c) * GW:(3 + c) * GW],
                        initial=gdf[:, c * G:c * G + 1], op0=OP.mult, op1=OP.add)
                # dU within segment
                du = pf.tile([P, 2 * G], F32, tag="du")
                uv = U[:].rearrange("p (c gw) -> p c gw", c=2)
                dv = du[:].rearrange("p (c g) -> p c g", c=2)
                nc.vector.tensor_tensor(out=dv[:, :, 0:G - 1], in0=uv[:, :, W + 1:W + G],
                                        in1=uv[:, :, W:W + G - 1], op=OP.subtract)
                d.update(A=A, gdf=gdf, cb=cb, U=U, du=du)

            def apply(s):
                d = st[s]
                U, du, A = d["U"], d["du"], d["A"]
                uv = U[:].rearrange("p (c gw) -> p c gw", c=2)
                # next-segment first U value (for the lerp across segment ends)
                unx = pf.tile([P, 2], F32, tag="unx")
                nc.scalar.dma_start(out=unx[0:P - 1, 0:2], in_=U[1:P, W::GW])
                if s < S - 1:
                    nc.scalar.dma_start(out=unx[P - 1:P, 0:2],
                                        in_=st[s + 1]["U"][0:1, W::GW])
                else:
                    nc.scalar.dma_start(out=unx[P - 1:P, 0:2],
                                        in_=U[P - 1:P, W + G - 1::GW])
                nc.vector.tensor_tensor(out=du[:, G - 1::G], in0=unx[:, 0:2],
                                        in1=U[:, W + G - 1::GW], op=OP.subtract)
                # upsample lerp: L[p, c, 16g+r] = U[g] + dU[g]*w0[r]
                L = pb.tile([P, 2 * M], F32, tag="L")
                lv = L[:].rearrange("p (c m) -> p c m", c=2)
                dv = du[:].rearrange("p (c g) -> p c g", c=2)
                for r in range(16):
                    nc.vector.scalar_tensor_tensor(
                        out=lv[:, :, r::16], in0=dv[:], scalar=w0[r],
                        in1=uv[:, :, W:W + G], op0=OP.mult, op1=OP.add)
                # exp + carrier multiply + store, split per channel so ACT/DVE/DMA
                # pipeline at half-chunk granularity
                for c in range(2):
                    lo, hi = c * M, (c + 1) * M
                    nc.scalar.activation(L[:, lo:hi], L[:, lo:hi], AF.Exp)
                    nc.vector.tensor_tensor(out=L[:, lo:hi], in0=A[:, lo:hi],
                                            in1=L[:, lo:hi], op=OP.mult)
                    nc.sync.dma_start(out=hbm_one(out, c, s), in_=L[:, lo:hi])

            for s in range(S):
                prep_scan(s)
                if s > 0:
                    apply(s - 1)
            apply(S - 1)

    nc.compile()
    return nc


def kernel(audio, threshold, ratio, makeup, attack_time, release_time):
    global LAST_RESULTS
    a = np.asarray(audio, dtype=np.float32)
    B, C, Tin = a.shape
    assert (B, C, Tin) == (B_TOTAL, 1, T), (B, C, Tin)
    thr = float(np.asarray(threshold).ravel()[0])
    rat = float(np.asarray(ratio).ravel()[0])
    mk = float(np.asarray(makeup).ravel()[0])
    at = float(np.asarray(attack_time).ravel()[0])
    rt = float(np.asarray(release_time).ravel()[0])

    nc = _build(thr, rat, mk, at, rt)

    flat = a.reshape(B_TOTAL, T)
    in_maps = [{"audio": np.ascontiguousarray(flat[i * NCH:(i + 1) * NCH])}
               for i in range(N_CORES)]
    res = run_bass_kernel_spmd(nc, in_maps, list(range(N_CORES)))
    LAST_RESULTS = res
    outp = np.concatenate([res.results[i]["out"] for i in range(N_CORES)], axis=0)
    return outp.reshape(B_TOTAL, 1, T).astype(np.float32)


# revision 6
# speedup vs baseline: 1.1352x; 1.0504x over previous
"""Trainium2 Bass kernel for nn_DynamicRangeCompressor.

Input : audio [16, 1, 2097152] f32 (+ scalar params threshold/ratio/makeup/
        attack_time/release_time as [1] arrays).
Output: [16, 1, 2097152] f32.

Sharding: pure data parallel - 2 batch rows per core across 8 NeuronCores.

Algorithm restructuring (validated vs reference to ~3e-7 rel err):
- Work in natural-log units (U = dB * ln10/20 + makeup_nat) so Ln/Exp replace
  log10/10**x and all scale factors fold away.
- linear_downsample(DS=16) == 0.5*(g[16i+7]+g[16i+8]): only 2/16 gain taps.
- The attack/release one-pole recurrence is branch-linearized: the coefficient
  is chosen by comparing gd[t] >= gd[t-1] instead of gd[t] >= y[t-1]. Since
  the coefficients are ~5e-5 the state tracks the target to ~3e-3 dB and the
  substitution changes the output by <1e-6 dB.  The scan becomes a LINEAR
  first-order IIR y = c[t]*y + b[t], which runs at stream rate on the DVE via
  tensor_tensor_scan(mult, add).
- Partition-parallel scan: each of 128 partitions scans its own time segment,
  seeded with W=4 warmup frames from the preceding segment (coefficient
  contraction (5.5e-5)^4 makes segments exactly independent in fp32).
- Hann overlap-add upsample == per-frame lerp: L[16q+r] = U[q] + dU[q]*w0[r],
  emitted as 16 strided scalar_tensor_tensor ops.
- out = audio * exp(L)  (drops reference's sign(a)*1e-8 term: |err| <= 1.5e-8).
"""
import os
import sys

for _p in ("/opt/trn_rl_repo", "/opt/pypackages"):
    if _p not in sys.path and os.path.isdir(_p):
        sys.path.append(_p)

import math
import numpy as np

import concourse.bass as bass
import concourse.tile as tile
from concourse import bacc, mybir
from concourse.bass_utils import run_bass_kernel_spmd

# problem constants (hardcoded per spec)
B_TOTAL = 16
T = 2097152
N_CORES = 8
NCH = 2               # batch rows per core
P = 128               # SBUF partitions
FD = T // P           # 16384 free-dim samples per partition per channel
S = 4                 # chunks per channel
M = FD // S           # samples per partition per chunk per channel
G = M // 16           # frames per partition per chunk per channel
W = 4                 # scan warmup frames
CS = P * M            # samples per chunk per channel
GW = G + W

F32 = mybir.dt.float32
OP = mybir.AluOpType
AF = mybir.ActivationFunctionType

LAST_RESULTS = None   # stashed BassKernelResults for test harness introspection

# Pin all activations to the one table set that contains Abs/Ln/Relu/Exp/
# Identity together (natural_log_exp_and_others); the default greedy set
# selection alternates between two sets and reloads tables 7x per run.
import concourse.bacc as _bacc_mod
from concourse.hw_specs import get_activation_tables as _real_gat


def _gat_pinned(arch):
    real = _real_gat(arch)
    return {name: (fns if name == "natural_log_exp_and_others" else set())
            for name, fns in real.items()}


_bacc_mod.get_activation_tables = _gat_pinned


def _build(thr, ratio, makeup, at, rt):
    ln10_20 = math.log(10.0) / 20.0
    thr_nat = float(np.float32(thr * ln10_20))
    mk_nat = float(np.float32(makeup * ln10_20))
    gscale = float(np.float32(-(1.0 - 1.0 / ratio) / 2.0))   # -0.375
    at = float(np.float32(at))
    rt = float(np.float32(rt))
    w0 = [float(0.5 * (1.0 - math.cos(2.0 * math.pi * r / 32.0))) for r in range(16)]

    nc = bacc.Bacc("TRN2", target_bir_lowering=False, debug=False)
    audio = nc.dram_tensor("audio", [NCH, T], F32, kind="ExternalInput")
    out = nc.dram_tensor("out", [NCH, T], F32, kind="ExternalOutput")

    def hbm_pair(dram, s):
        # [128, 2, M]: partition-major chunk, both channels in one DMA
        return dram[:, s * CS:(s + 1) * CS].rearrange("c (p m) -> p c m", p=P)

    def hbm_one(dram, c, s):
        return dram[c:c + 1, s * CS:(s + 1) * CS].rearrange(
            "one (p m) -> (one p) m", p=P)

    with tile.TileContext(nc) as tc:
        with tc.tile_pool(name="big", bufs=2) as pb, \
             tc.tile_pool(name="fr", bufs=2) as pf, \
             tc.tile_pool(name="consts", bufs=1) as pc:

            bias_eps = pc.tile([P, 1], F32, tag="bias_eps")
            bias_nthr = pc.tile([P, 1], F32, tag="bias_nthr")
            bias_mk = pc.tile([P, 1], F32, tag="bias_mk")
            bias_rt = pc.tile([P, 1], F32, tag="bias_rt")
            bias_omrt = pc.tile([P, 1], F32, tag="bias_omrt")
            nc.vector.memset(bias_eps[:], 1e-8)
            nc.vector.memset(bias_nthr[:], -thr_nat)
            nc.vector.memset(bias_mk[:], mk_nat)
            nc.vector.memset(bias_rt[:], rt)
            nc.vector.memset(bias_omrt[:], 1.0 - rt)

            st = [{} for _ in range(S)]  # per-chunk tiles

            def prep_scan(s):
                d = st[s]
                A = pb.tile([P, 2 * M], F32, tag="A")
                av = A[:].rearrange("p (c m) -> p c m", c=2)
                nc.sync.dma_start(out=av[:], in_=hbm_pair(audio, s))

                t7 = pf.tile([P, 2 * G], F32, tag="t7")
                t8 = pf.tile([P, 2 * G], F32, tag="t8")
                for t_, off in ((t7, 7), (t8, 8)):
                    nc.scalar.activation(t_[:], A[:, off::16], AF.Abs)
                    nc.scalar.activation(t_[:], t_[:], AF.Ln, bias=bias_eps[:])
                    nc.scalar.activation(t_[:], t_[:], AF.Relu, bias=bias_nthr[:])
                # gd in nat units incl makeup: gdf = gscale*(t7+t8) + mk_nat
                # laid out [prev | ch0 frames | prev | ch1 frames] (G+1 per ch)
                # so the >= compare needs no separate boundary op
                nc.scalar.activation(t7[:], t7[:], AF.Identity, bias=bias_mk[:],
                                     scale=gscale)
                gdf = pf.tile([P, 2 * (G + 1)], F32, tag="gdf")
                gx = gdf[:].rearrange("p (c g1) -> p c g1", c=2)
                nc.vector.scalar_tensor_tensor(
                    out=gx[:, :, 1:G + 1],
                    in0=t8[:].rearrange("p (c g) -> p c g", c=2), scalar=gscale,
                    in1=t7[:].rearrange("p (c g) -> p c g", c=2),
                    op0=OP.mult, op1=OP.add)
                # boundary col 0 of each channel = previous segment's last frame
                nc.sync.dma_start(out=gdf[1:P, 0::G + 1], in_=gdf[0:P - 1, G::G + 1])
                if s > 0:
                    nc.sync.dma_start(out=gdf[0:1, 0::G + 1],
                                      in_=st[s - 1]["gdf"][P - 1:P, G::G + 1])
                else:
                    nc.sync.dma_start(out=gdf[0:1, 0::G + 1], in_=gdf[0:1, 1::G + 1])
                mt = pf.tile([P, 2 * G], F32, tag="mt")
                mv = mt[:].rearrange("p (c g) -> p c g", c=2)
                nc.vector.tensor_tensor(out=mv[:], in0=gx[:, :, 1:G + 1],
                                        in1=gx[:, :, 0:G], op=OP.is_ge)
                # coefficients, written into the c|b ext array
                # layout: [c_ch0 | c_ch1 | b_ch0 | b_ch1], each GW wide
                cb = pf.tile([P, 4 * GW], F32, tag="cb")
                cbv = cb[:].rearrange("p (h gw) -> p h gw", h=4)
                # c = m*(at-rt) + rt   (ACT)
                nc.scalar.activation(cbv[:, 0:2, W:GW], mv[:], AF.Identity,
                                     bias=bias_rt[:], scale=at - rt)
                # omc = m*(rt-at) + (1-rt)   (ACT, in-place over m)
                nc.scalar.activation(mt[:], mt[:], AF.Identity,
                                     bias=bias_omrt[:], scale=rt - at)
                # b = omc * gd
                nc.vector.tensor_tensor(out=cbv[:, 2:4, W:GW], in0=mv[:],
                                        in1=gx[:, :, 1:G + 1], op=OP.mult)
                # warmup columns = last W real frames of the preceding segment
                nc.sync.dma_start(out=cbv[1:P, :, 0:W], in_=cbv[0:P - 1, :, G:GW])
                if s > 0:
                    pcb = st[s - 1]["cb"][:].rearrange("p (h gw) -> p h gw", h=4)
                    nc.sync.dma_start(out=cbv[0:1, :, 0:W],
                                      in_=pcb[P - 1:P, :, G:GW])
                else:
                    nc.vector.memset(cbv[0:1, 0:2, 0:W], 1.0)
                    nc.vector.memset(cbv[0:1, 2:4, 0:W], 0.0)
                # the scan: one linear IIR per partition per channel.
                # U layout per channel: [W warmup | G real | 1 next-seg-first]
                GW1 = GW + 1
                U = pf.tile([P, 2 * GW1], F32, tag="U")
                for c in range(2):
                    nc.vector.tensor_tensor_scan(
                        out=U[:, c * GW1:c * GW1 + GW],
                        data0=cb[:, c * GW:(c + 1) * GW],
                        data1=cb[:, (2 + c) * GW:(3 + c) * GW],
                        initial=gdf[:, c * (G + 1) + 1:c * (G + 1) + 2],
                        op0=OP.mult, op1=OP.add)
                d.update(A=A, gdf=gdf, cb=cb, U=U)

            def apply(s):
                d = st[s]
                U, A = d["U"], d["A"]
                GW1 = GW + 1
                uv = U[:].rearrange("p (c gw1) -> p c gw1", c=2)
                # next-segment first U value into col GW of each channel block
                nc.sync.dma_start(out=U[0:P - 1, GW::GW1], in_=U[1:P, W::GW1])
                if s < S - 1:
                    nc.sync.dma_start(out=U[P - 1:P, GW::GW1],
                                      in_=st[s + 1]["U"][0:1, W::GW1])
                else:
                    nc.sync.dma_start(out=U[P - 1:P, GW::GW1],
                                      in_=U[P - 1:P, W + G - 1::GW1])
                # dU (incl. cross-segment last column, in one op)
                du = pf.tile([P, 2 * G], F32, tag="du")
                dv = du[:].rearrange("p (c g) -> p c g", c=2)
                nc.vector.tensor_tensor(out=dv[:], in0=uv[:, :, W + 1:W + G + 1],
                                        in1=uv[:, :, W:W + G], op=OP.subtract)
                # upsample lerp: L[p, c, 16g+r] = U[g] + dU[g]*w0[r]
                L = pb.tile([P, 2 * M], F32, tag="L")
                lv = L[:].rearrange("p (c m) -> p c m", c=2)
                for r in range(16):
                    nc.vector.scalar_tensor_tensor(
                        out=lv[:, :, r::16], in0=dv[:], scalar=w0[r],
                        in1=uv[:, :, W:W + G], op0=OP.mult, op1=OP.add)
                # exp + carrier multiply + store, split per channel so ACT/DVE/DMA
                # pipeline at half-chunk granularity
                for c in range(2):
                    lo, hi = c * M, (c + 1) * M
                    nc.scalar.activation(L[:, lo:hi], L[:, lo:hi], AF.Exp)
                    nc.vector.tensor_tensor(out=L[:, lo:hi], in0=A[:, lo:hi],
                                            in1=L[:, lo:hi], op=OP.mult)
                    nc.sync.dma_start(out=hbm_one(out, c, s), in_=L[:, lo:hi])

            for s in range(S):
                prep_scan(s)
                if s > 0:
                    apply(s - 1)
            apply(S - 1)

    nc.compile()
    return nc


def kernel(audio, threshold, ratio, makeup, attack_time, release_time):
    global LAST_RESULTS
    a = np.asarray(audio, dtype=np.float32)
    B, C, Tin = a.shape
    assert (B, C, Tin) == (B_TOTAL, 1, T), (B, C, Tin)
    thr = float(np.asarray(threshold).ravel()[0])
    rat = float(np.asarray(ratio).ravel()[0])
    mk = float(np.asarray(makeup).ravel()[0])
    at = float(np.asarray(attack_time).ravel()[0])
    rt = float(np.asarray(release_time).ravel()[0])

    nc = _build(thr, rat, mk, at, rt)

    flat = a.reshape(B_TOTAL, T)
    in_maps = [{"audio": np.ascontiguousarray(flat[i * NCH:(i + 1) * NCH])}
               for i in range(N_CORES)]
    res = run_bass_kernel_spmd(nc, in_maps, list(range(N_CORES)))
    LAST_RESULTS = res
    outp = np.concatenate([res.results[i]["out"] for i in range(N_CORES)], axis=0)
    return outp.reshape(B_TOTAL, 1, T).astype(np.float32)


# revision 10
# speedup vs baseline: 1.2060x; 1.0624x over previous
"""Trainium2 Bass kernel for nn_DynamicRangeCompressor.

Input : audio [16, 1, 2097152] f32 (+ scalar params threshold/ratio/makeup/
        attack_time/release_time as [1] arrays).
Output: [16, 1, 2097152] f32.

Sharding: pure data parallel - 2 batch rows per core across 8 NeuronCores.

Algorithm restructuring (validated vs reference to ~3e-7 rel err):
- Work in natural-log units (U = dB * ln10/20 + makeup_nat) so Ln/Exp replace
  log10/10**x and all scale factors fold away.
- linear_downsample(DS=16) == 0.5*(g[16i+7]+g[16i+8]): only 2/16 gain taps.
- The attack/release one-pole recurrence is branch-linearized: the coefficient
  is chosen by comparing gd[t] >= gd[t-1] instead of gd[t] >= y[t-1]. Since
  the coefficients are ~5e-5 the state tracks the target to ~3e-3 dB and the
  substitution changes the output by <1e-6 dB.  The scan becomes a LINEAR
  first-order IIR y = c[t]*y + b[t], which runs at stream rate on the DVE via
  tensor_tensor_scan(mult, add).
- Partition-parallel scan: each of 128 partitions scans its own time segment,
  seeded with W=4 warmup frames from the preceding segment (coefficient
  contraction (5.5e-5)^4 makes segments exactly independent in fp32).
- Hann overlap-add upsample == per-frame lerp: L[16q+r] = U[q] + dU[q]*w0[r],
  emitted as 16 strided scalar_tensor_tensor ops.
- out = audio * exp(L)  (drops reference's sign(a)*1e-8 term: |err| <= 1.5e-8).
"""
import os
import sys

for _p in ("/opt/trn_rl_repo", "/opt/pypackages"):
    if _p not in sys.path and os.path.isdir(_p):
        sys.path.append(_p)

import math
import numpy as np

import concourse.bass as bass
import concourse.tile as tile
from concourse import bacc, mybir
from concourse.bass_utils import run_bass_kernel_spmd

# problem constants (hardcoded per spec)
B_TOTAL = 16
T = 2097152
N_CORES = 8
NCH = 2               # batch rows per core
P = 128               # SBUF partitions
FD = T // P           # 16384 free-dim samples per partition per channel
S = 4                 # chunks per channel
M = FD // S           # samples per partition per chunk per channel
G = M // 16           # frames per partition per chunk per channel
W = 4                 # scan warmup frames
CS = P * M            # samples per chunk per channel
GW = G + W

F32 = mybir.dt.float32
OP = mybir.AluOpType
AF = mybir.ActivationFunctionType

LAST_RESULTS = None   # stashed BassKernelResults for test harness introspection

# Pin all activations to the one table set that contains Abs/Ln/Relu/Exp/
# Identity together (natural_log_exp_and_others); the default greedy set
# selection alternates between two sets and reloads tables 7x per run.
import concourse.bacc as _bacc_mod
from concourse.hw_specs import get_activation_tables as _real_gat


def _gat_pinned(arch):
    real = _real_gat(arch)
    return {name: (fns if name == "natural_log_exp_and_others" else set())
            for name, fns in real.items()}


_bacc_mod.get_activation_tables = _gat_pinned


def _build(thr, ratio, makeup, at, rt):
    ln10_20 = math.log(10.0) / 20.0
    thr_nat = float(np.float32(thr * ln10_20))
    mk_nat = float(np.float32(makeup * ln10_20))
    gscale = float(np.float32(-(1.0 - 1.0 / ratio) / 2.0))   # -0.375
    at = float(np.float32(at))
    rt = float(np.float32(rt))
    w0 = [float(0.5 * (1.0 - math.cos(2.0 * math.pi * r / 32.0))) for r in range(16)]

    nc = bacc.Bacc("TRN2", target_bir_lowering=False, debug=False)
    audio = nc.dram_tensor("audio", [NCH, T], F32, kind="ExternalInput")
    out = nc.dram_tensor("out", [NCH, T], F32, kind="ExternalOutput")

    def hbm_pair(dram, s):
        # [128, 2, M]: partition-major chunk, both channels in one DMA
        return dram[:, s * CS:(s + 1) * CS].rearrange("c (p m) -> p c m", p=P)

    def hbm_one(dram, c, s):
        return dram[c:c + 1, s * CS:(s + 1) * CS].rearrange(
            "one (p m) -> (one p) m", p=P)

    with tile.TileContext(nc) as tc:
        with tc.tile_pool(name="aud", bufs=3) as pa, \
             tc.tile_pool(name="big", bufs=2) as pb, \
             tc.tile_pool(name="fr", bufs=2) as pf, \
             tc.tile_pool(name="consts", bufs=1) as pc:

            bias_eps = pc.tile([P, 1], F32, tag="bias_eps")
            bias_nthr = pc.tile([P, 1], F32, tag="bias_nthr")
            bias_mk = pc.tile([P, 1], F32, tag="bias_mk")
            bias_rt = pc.tile([P, 1], F32, tag="bias_rt")
            bias_omrt = pc.tile([P, 1], F32, tag="bias_omrt")
            nc.vector.memset(bias_eps[:], 1e-8)
            nc.vector.memset(bias_nthr[:], -thr_nat)
            nc.vector.memset(bias_mk[:], mk_nat)
            nc.vector.memset(bias_rt[:], rt)
            nc.vector.memset(bias_omrt[:], 1.0 - rt)
            w0t = pc.tile([P, 16], F32, tag="w0t")
            for r in range(16):
                nc.vector.memset(w0t[:, r:r + 1], w0[r])

            st = [{} for _ in range(S)]  # per-chunk tiles

            def prep_scan(s):
                d = st[s]
                A = pa.tile([P, 2 * M], F32, tag="A")
                av = A[:].rearrange("p (c m) -> p c m", c=2)
                nc.sync.dma_start(out=av[:], in_=hbm_pair(audio, s))

                t7 = pf.tile([P, 2 * G], F32, tag="t7")
                t8 = pf.tile([P, 2 * G], F32, tag="t8")
                for t_, off in ((t7, 7), (t8, 8)):
                    nc.scalar.activation(t_[:], A[:, off::16], AF.Abs)
                    nc.scalar.activation(t_[:], t_[:], AF.Ln, bias=bias_eps[:])
                    nc.scalar.activation(t_[:], t_[:], AF.Relu, bias=bias_nthr[:])
                # gd in nat units incl makeup: gdf = gscale*(t7+t8) + mk_nat
                # laid out [prev | ch0 frames | prev | ch1 frames] (G+1 per ch)
                # so the >= compare needs no separate boundary op
                nc.scalar.activation(t7[:], t7[:], AF.Identity, bias=bias_mk[:],
                                     scale=gscale)
                gdf = pf.tile([P, 2 * (G + 1)], F32, tag="gdf")
                gx = gdf[:].rearrange("p (c g1) -> p c g1", c=2)
                nc.vector.scalar_tensor_tensor(
                    out=gx[:, :, 1:G + 1],
                    in0=t8[:].rearrange("p (c g) -> p c g", c=2), scalar=gscale,
                    in1=t7[:].rearrange("p (c g) -> p c g", c=2),
                    op0=OP.mult, op1=OP.add)
                # boundary col 0 of each channel = previous segment's last frame
                nc.sync.dma_start(out=gdf[1:P, 0::G + 1], in_=gdf[0:P - 1, G::G + 1])
                if s > 0:
                    nc.sync.dma_start(out=gdf[0:1, 0::G + 1],
                                      in_=st[s - 1]["gdf"][P - 1:P, G::G + 1])
                else:
                    nc.sync.dma_start(out=gdf[0:1, 0::G + 1], in_=gdf[0:1, 1::G + 1])
                mt = pf.tile([P, 2 * G], F32, tag="t7")
                mv = mt[:].rearrange("p (c g) -> p c g", c=2)
                nc.vector.tensor_tensor(out=mv[:], in0=gx[:, :, 1:G + 1],
                                        in1=gx[:, :, 0:G], op=OP.is_ge)
                # coefficients, written into the c|b ext array
                # layout: [c_ch0 | c_ch1 | b_ch0 | b_ch1], each GW wide
                cb = pf.tile([P, 4 * GW], F32, tag="cb")
                cbv = cb[:].rearrange("p (h gw) -> p h gw", h=4)
                # c = m*(at-rt) + rt   (ACT)
                nc.scalar.activation(cbv[:, 0:2, W:GW], mv[:], AF.Identity,
                                     bias=bias_rt[:], scale=at - rt)
                # omc = m*(rt-at) + (1-rt)   (ACT, in-place over m)
                nc.scalar.activation(mt[:], mt[:], AF.Identity,
                                     bias=bias_omrt[:], scale=rt - at)
                # b = omc * gd
                nc.vector.tensor_tensor(out=cbv[:, 2:4, W:GW], in0=mv[:],
                                        in1=gx[:, :, 1:G + 1], op=OP.mult)
                # warmup columns = last W real frames of the preceding segment
                nc.sync.dma_start(out=cbv[1:P, :, 0:W], in_=cbv[0:P - 1, :, G:GW])
                if s > 0:
                    pcb = st[s - 1]["cb"][:].rearrange("p (h gw) -> p h gw", h=4)
                    nc.sync.dma_start(out=cbv[0:1, :, 0:W],
                                      in_=pcb[P - 1:P, :, G:GW])
                else:
                    nc.vector.memset(cbv[0:1, 0:2, 0:W], 1.0)
                    nc.vector.memset(cbv[0:1, 2:4, 0:W], 0.0)
                # the scan: one linear IIR per partition per channel.
                # U layout per channel: [W warmup | G real | 1 next-seg-first]
                GW1 = GW + 1
                U = pf.tile([P, 2 * GW1], F32, tag="U")
                for c in range(2):
                    nc.vector.tensor_tensor_scan(
                        out=U[:, c * GW1:c * GW1 + GW],
                        data0=cb[:, c * GW:(c + 1) * GW],
                        data1=cb[:, (2 + c) * GW:(3 + c) * GW],
                        initial=gdf[:, c * (G + 1) + 1:c * (G + 1) + 2],
                        op0=OP.mult, op1=OP.add)
                d.update(A=A, gdf=gdf, cb=cb, U=U)

            def apply(s):
                # bulk of the chunk: the last frame of every segment is computed
                # with a stale dU and patched in fixup(s) (which needs chunk s+1's
                # scan); everything here depends only on chunk s.
                d = st[s]
                U, A = d["U"], d["A"]
                GW1 = GW + 1
                uv = U[:].rearrange("p (c gw1) -> p c gw1", c=2)
                du = pf.tile([P, 2 * G], F32, tag="t8")
                dv = du[:].rearrange("p (c g) -> p c g", c=2)
                nc.vector.tensor_tensor(out=dv[:, :, 0:G - 1],
                                        in0=uv[:, :, W + 1:W + G],
                                        in1=uv[:, :, W:W + G - 1], op=OP.subtract)
                # upsample lerp: L[p, c, 16g+r] = U[g] + dU[g]*w0[r]
                L = pb.tile([P, 2 * M], F32, tag="L")
                lv = L[:].rearrange("p (c m) -> p c m", c=2)
                for r in range(16):
                    nc.vector.scalar_tensor_tensor(
                        out=lv[:, :, r::16], in0=dv[:], scalar=w0[r],
                        in1=uv[:, :, W:W + G], op0=OP.mult, op1=OP.add)
                for c in range(2):
                    lo, hi = c * M, (c + 1) * M
                    nc.scalar.activation(L[:, lo:hi], L[:, lo:hi], AF.Exp)
                    nc.vector.tensor_tensor(out=L[:, lo:hi], in0=A[:, lo:hi],
                                            in1=L[:, lo:hi], op=OP.mult)
                d["L"] = L

            def fixup(s):
                # patch the 16 last-frame samples of every segment, then store
                d = st[s]
                U, A, L = d["U"], d["A"], d["L"]
                GW1 = GW + 1
                uv = U[:].rearrange("p (c gw1) -> p c gw1", c=2)
                lv = L[:].rearrange("p (c m) -> p c m", c=2)
                # next-segment first U value into col GW of each channel block
                nc.sync.dma_start(out=U[0:P - 1, GW::GW1], in_=U[1:P, W::GW1])
                if s < S - 1:
                    nc.sync.dma_start(out=U[P - 1:P, GW::GW1],
                                      in_=st[s + 1]["U"][0:1, W::GW1])
                else:
                    nc.sync.dma_start(out=U[P - 1:P, GW::GW1],
                                      in_=U[P - 1:P, W + G - 1::GW1])
                dul = pf.tile([P, 2], F32, tag="dul")
                nc.vector.tensor_tensor(out=dul[:], in0=U[:, GW::GW1],
                                        in1=U[:, W + G - 1::GW1], op=OP.subtract)
                for c in range(2):
                    sl = lv[:, c, M - 16:M]
                    nc.vector.tensor_scalar(
                        out=sl, in0=w0t[:], scalar1=dul[:, c:c + 1],
                        scalar2=uv[:, c, W + G - 1:W + G],
                        op0=OP.mult, op1=OP.add)
                    nc.scalar.activation(sl, sl, AF.Exp)
                    nc.vector.tensor_tensor(
                        out=sl, in0=A[:].rearrange("p (c m) -> p c m", c=2)[:, c, M - 16:M],
                        in1=sl, op=OP.mult)
                    nc.sync.dma_start(out=hbm_one(out, c, s),
                                      in_=L[:, c * M:(c + 1) * M])

            for s in range(S):
                prep_scan(s)
                apply(s)
                if s > 0:
                    fixup(s - 1)
            fixup(S - 1)

    nc.compile()
    return nc


def kernel(audio, threshold, ratio, makeup, attack_time, release_time):
    global LAST_RESULTS
    a = np.asarray(audio, dtype=np.float32)
    B, C, Tin = a.shape
    assert (B, C, Tin) == (B_TOTAL, 1, T), (B, C, Tin)
    thr = float(np.asarray(threshold).ravel()[0])
    rat = float(np.asarray(ratio).ravel()[0])
    mk = float(np.asarray(makeup).ravel()[0])
    at = float(np.asarray(attack_time).ravel()[0])
    rt = float(np.asarray(release_time).ravel()[0])

    nc = _build(thr, rat, mk, at, rt)

    flat = a.reshape(B_TOTAL, T)
    in_maps = [{"audio": np.ascontiguousarray(flat[i * NCH:(i + 1) * NCH])}
               for i in range(N_CORES)]
    res = run_bass_kernel_spmd(nc, in_maps, list(range(N_CORES)))
    LAST_RESULTS = res
    outp = np.concatenate([res.results[i]["out"] for i in range(N_CORES)], axis=0)
    return outp.reshape(B_TOTAL, 1, T).astype(np.float32)
